# revision 38
# baseline (speedup 1.0000x reference)
"""DualAttention Trainium2 kernel (8 NeuronCores, data-parallel over batch).

Math (per batch b, head h, dk=64, S=1024):
  s   = (q @ k^T) / 8                      [S, S]
  E   = exp(s) with strict-causal mask (j < i) applied as -1e30 pre-exp
  Z1  = rowsum(E)                          (row 0: Z1=0 -> handled specially)
  p   = E / Z1                             (NO counter-mask here -- see below)
  E2  = exp(p * notcm) dense over full S; using exp(0)=1 off-window:
    out*Z2 = exp(p)|W @ (notcm*v)|W + [colsumAll(v) - colsumW(notcm*v)]
    Z2     = exp(p)|W @ notcm|W     + [S - countW(notcm)]
  i.e. the counter-mask commutes onto v (host-precomputed vx = notcm*v with
  an appended notcm column for Z2), and the "+1 everywhere" part of E2
  becomes a host-precomputed per-(head, qb) rank-1 correction row.
  row 0 of out is forced to 0 (reference zeroes p row 0 post-softmax).

Kernel per core (1 batch item): loop 16 heads; per head: scores via PE
(fp16), causal -1e30 via a const matmul addend, exp1 on ACT with fused
accum -> Z1, E*(1/Z1) on DVE tensor_scalar, batched exp2 on ACT,
DMA-xbar transposes of E2 (bf16), one N=130 matmul per (qb,kc) chunk
against [v1|v2|notcm] plus one fp16 rank-1 corr matmul per qb, 1/Z2 via
a strided psum read, bf16 outputs (host upcasts).
"""

import numpy as np

import concourse.bass as bass
import concourse.mybir as mybir
from concourse.tile import TileContext
from concourse.alu_op_type import AluOpType

F32 = mybir.dt.float32
BF16 = mybir.dt.bfloat16
FP16 = mybir.dt.float16

B, S, D = 8, 1024, 1024
H, DK = 16, 64
NCORES = 8
P = 128          # partition block
NQB = S // P     # 8 query blocks
MASKADD = -1e30
HV = 130         # per-head vx cols: v1(64) | v2(64) | notcm(1) | pad(1)
# packed offsets for the causal windows W=(qb+1)*128
OFF = [0]
for _qb in range(NQB):
    OFF.append(OFF[-1] + (_qb + 1) * P)
TOTW = OFF[-1]   # 4608
POB = 256        # psum cols per qb block in po (130 used)


def build_nc():
    from concourse.bacc import Bacc

    nc = Bacc()
    # host passes q/k pre-transposed [D, S] fp16; vx = notcm*[v1|v2|notcm]
    # interleaved per head (bf16); corr = per-(qb, head) rank-1 rows (fp16)
    qt_d = nc.declare_dram_parameter("qT", [D, S], FP16, isOutput=False)
    kt_d = nc.declare_dram_parameter("kT", [D, S], FP16, isOutput=False)
    vx_d = nc.declare_dram_parameter("vx", [S, H * HV], BF16, isOutput=False)
    cr_d = nc.declare_dram_parameter("corr", [1, NQB * H * HV], FP16,
                                     isOutput=False)
    o1_d = nc.declare_dram_parameter("out1", [S, D], BF16, isOutput=True)
    o2_d = nc.declare_dram_parameter("out2", [S, D], BF16, isOutput=True)

    from contextlib import ExitStack

    with TileContext(nc) as tc, ExitStack() as ctx:
        const = ctx.enter_context(tc.tile_pool(name="const", bufs=1))
        qkpool = ctx.enter_context(tc.tile_pool(name="qk", bufs=2))
        vpool = ctx.enter_context(tc.tile_pool(name="vp", bufs=4))
        epool = ctx.enter_context(tc.tile_pool(name="ep", bufs=16))
        packp = ctx.enter_context(tc.tile_pool(name="pk", bufs=2))
        etp = ctx.enter_context(tc.tile_pool(name="et", bufs=3))
        smol = ctx.enter_context(tc.tile_pool(name="sm", bufs=6))
        outp = ctx.enter_context(tc.tile_pool(name="op", bufs=2))
        bigp = ctx.enter_context(tc.tile_pool(name="big", bufs=1))
        # PSUM budget (8 banks): ps 2x2 + po 1x4
        ps_pool = ctx.enter_context(tc.tile_pool(name="ps", bufs=2, space="PSUM"))
        po_pool = ctx.enter_context(tc.tile_pool(name="po", bufs=1, space="PSUM"))

        # ---------------- constants ----------------
        # touch Exp immediately so the ~2.7us ACT table load overlaps the
        # first input DMAs instead of stalling the first exp1
        warm = const.tile([1, 1], F32, tag="warm")
        nc.gpsimd.memset(warm[:], 0.0)
        nc.scalar.activation(out=warm[:], in_=warm[:],
                             func=mybir.ActivationFunctionType.Exp)

        ones_row = const.tile([1, P], FP16, tag="onesrow")
        nc.gpsimd.memset(ones_row[:], 1.0)

        corr_sb = const.tile([1, NQB * H * HV], FP16, tag="corr")
        nc.sync.dma_start(out=corr_sb[:], in_=cr_d[:])

        # ------------- main loop: 16 heads, 3-stage software pipeline ------
        # A(h): scores + causal + exp1 (+loads). B(h): 1/Z1, scale,
        # exp2, transpose, P@V. C(h): 1/Z2, scale, store. Emitting
        # A(h), C(h-2), B(h-1) keeps each engine's FIFO free of stalls.
        state = {}
        # full outputs accumulate in SBUF; flushed in 512B-run DMAs per
        # 4-head group (strided shorter writes are ~4x slower)
        big1 = bigp.tile([P, NQB * S], BF16, tag="big1")
        big2 = bigp.tile([P, NQB * S], BF16, tag="big2")

        def stage_load(hp):
            if hp >= NQB or ("pair", hp) in state:
                return
            dsl = slice(hp * P, (hp + 1) * P)
            qT2 = qkpool.tile([P, S], FP16, tag="qT2")
            kT2 = qkpool.tile([P, S], FP16, tag="kT2")
            nc.sync.dma_start(out=qT2[:], in_=qt_d[dsl, :])
            nc.sync.dma_start(out=kT2[:], in_=kt_d[dsl, :])
            # vx tile: SBUF[p, (c, d)] = DRAM[c*128+p, pair cols], one DMA
            vxt = vpool.tile([P, NQB * 2 * HV], BF16, tag="vx")
            vsl = slice(hp * 2 * HV, (hp + 1) * 2 * HV)
            nc.sync.dma_start(
                out=vxt.rearrange("p (c d) -> p c d", c=NQB),
                in_=vx_d[:, vsl].rearrange("(c s) d -> s c d", c=NQB))
            state[("pair", hp)] = (qT2, kT2, vxt)

        def stage_a(h):
            hp, hl = divmod(h, 2)
            stage_load(hp)
            z1 = smol.tile([P, NQB], F32, tag="z1")
            state[h] = dict(z1=z1, etiles=[])
            _score_exp1(h, range(0, NQB // 2))

        def _score_exp1(h, qbs):
            st = state[h]
            hp, hl = divmod(h, 2)
            qT2, kT2, _ = state[("pair", hp)]
            pb = hl * DK
            z1, etiles = st["z1"], st["etiles"]
            for qb in qbs:
                W = (qb + 1) * P
                ps = ps_pool.tile([P, S], F32, tag="ps")
                # split at the 512-col psum bank boundary
                for lo in range(0, W, 512):
                    hi = min(lo + 512, W)
                    nc.tensor.matmul(
                        ps[:, lo:hi],
                        qT2[pb : pb + DK, qb * P : (qb + 1) * P],
                        kT2[pb : pb + DK, lo:hi],
                        start=True, stop=True)
                # exp of UNMASKED scores; strict-causal zeroing of the
                # diagonal block happens post-exp (gpsimd affine_select
                # keeps c < r, fills 0), then Z1 = rowsum on DVE. Keeps
                # the tric addend matmuls off the tensor engine and the
                # accumulator drains off the scalar engine.
                e_t = epool.tile([P, S], BF16, tag="E")
                nc.scalar.activation(
                    out=e_t[:, 0:W], in_=ps[:, 0:W],
                    func=mybir.ActivationFunctionType.Exp,
                    scale=0.125)
                with tc.high_priority():
                    nc.gpsimd.affine_select(
                        out=e_t[:, W - P : W], in_=e_t[:, W - P : W],
                        compare_op=AluOpType.is_ge,
                        fill=0.0, base=-1, pattern=[[-1, P]],
                        channel_multiplier=1)
                nc.vector.tensor_reduce(
                    out=z1[:, qb : qb + 1], in_=e_t[:, 0:W],
                    axis=mybir.AxisListType.X, op=AluOpType.add)
                etiles.append(e_t)

        def stage_a2(h):
            _score_exp1(h, range(NQB // 2, NQB))

        def stage_b1(h):
            st = state[h]
            r1 = smol.tile([P, NQB], F32, tag="r1")
            nc.vector.reciprocal(r1[:], st["z1"][:])
            # query row 0 has Z1=0; force scale 0 (out row zeroed later)
            nc.gpsimd.memset(r1[0:1, 0:1], 0.0)

            # exp2 = exp(E * 1/Z1): the 1/Z1 folds into the ACT's
            # per-partition scale, so the DVE never touches these cols
            pp = packp.tile([P, TOTW], BF16, tag="pp")
            for qb in range(NQB):
                W = (qb + 1) * P
                nc.scalar.activation(
                    out=pp[:, OFF[qb] : OFF[qb] + W],
                    in_=st["etiles"][qb][:, 0:W],
                    func=mybir.ActivationFunctionType.Exp,
                    scale=r1[:, qb : qb + 1])

            # all 36 (qb, kc) chunks transposed in two blocked DMAs; P@V
            # consumes them only at the NEXT iteration (pipeline distance
            # 2), so the xbar latency hides behind the next head's scores
            e2t = etp.tile([P, TOTW], BF16, tag="e2t")
            NB4 = OFF[4] // P  # 10 chunks in qb 0..3
            nc.sync.dma_start(
                out=e2t[:, 0 : OFF[4]].rearrange("p (n s) -> p n s", n=NB4),
                in_=pp[:, 0 : OFF[4]].rearrange("p (n s) -> p n s", n=NB4),
                transpose=True)
            nc.sync.dma_start(
                out=e2t[:, OFF[4] :].rearrange("p (n s) -> p n s",
                                               n=TOTW // P - NB4),
                in_=pp[:, OFF[4] :].rearrange("p (n s) -> p n s",
                                              n=TOTW // P - NB4),
                transpose=True)
            st["e2t"] = e2t

        def stage_pv(h):
            st = state[h]
            hp, hl = divmod(h, 2)
            _, _, vxt = state[("pair", hp)]
            e2t = st["e2t"]

            # P@[v1|v2|notcm] per chunk + rank-1 corr row per qb.
            # po: 4 banks, block qb at POB*qb (130 of 256 cols used).
            po = po_pool.tile([P, NQB * POB], F32, tag="po")
            for qb in range(NQB):
                ob = qb * POB
                nc.tensor.matmul(
                    po[:, ob : ob + HV],
                    ones_row[:],
                    corr_sb[0:1, (qb * H + h) * HV : (qb * H + h + 1) * HV],
                    start=(qb % 2 == 0), stop=False)
                for kc in range(qb + 1):
                    n = OFF[qb] // P + kc
                    nc.tensor.matmul(
                        po[:, ob : ob + HV],
                        e2t[:, n * P : (n + 1) * P],
                        vxt[:, kc * 2 * HV + hl * HV : kc * 2 * HV + (hl + 1) * HV],
                        start=False,
                        stop=(qb % 2 == 1 and kc == qb))
            st["po"] = po

        def stage_c(h):
            st = state.pop(h)
            po = st["po"]
            po3 = po.rearrange("p (c x) -> p c x", c=NQB)
            r2 = smol.tile([P, NQB], F32, tag="r2")
            nc.vector.reciprocal(r2[:], po3[:, :, 128:129])

            obuf = outp.tile([P, S], BF16, tag="osb")
            for qb in range(NQB):
                nc.vector.tensor_scalar_mul(
                    obuf[:, qb * P : (qb + 1) * P],
                    po[:, qb * POB : qb * POB + P],
                    r2[:, qb : qb + 1])
            # spread into the big output accumulators (DVE copies bf16 at
            # 4 elem/cycle/lane -- ~6x faster than the gpsimd Q7 path,
            # and it keeps the spread out of the gpsimd FIFO where it
            # was delaying the critical-path affine_selects)
            ob3 = obuf.rearrange("p (c x) -> p c x", c=NQB)
            b13 = big1.rearrange("p (c d) -> p c d", c=NQB)
            b23 = big2.rearrange("p (c d) -> p c d", c=NQB)
            hc = slice(h * DK, (h + 1) * DK)
            nc.vector.tensor_copy(b13[:, :, hc], ob3[:, :, 0:DK])
            nc.vector.tensor_copy(b23[:, :, hc], ob3[:, :, DK:P])
            nc.gpsimd.memset(big1[0:1, h * DK : (h + 1) * DK], 0.0)
            nc.gpsimd.memset(big2[0:1, h * DK : (h + 1) * DK], 0.0)
            if h % 4 == 3:
                g = slice((h - 3) * DK, (h + 1) * DK)
                nc.sync.dma_start(
                    out=o1_d[:, g].rearrange("(c s) d -> s c d", c=NQB),
                    in_=b13[:, :, g])
                nc.sync.dma_start(
                    out=o2_d[:, g].rearrange("(c s) d -> s c d", c=NQB),
                    in_=b23[:, :, g])

        # PV at distance 3: its e2t transposes complete a full iteration
        # before PV enters the tensor FIFO, so the PE never head-of-line
        # blocks on the xbar (a >3us PE gap drops the HAM clock 2.4->1.2
        # GHz, doubling every matmul's column time)
        for it in range(H + 4):
            if it < H:
                stage_a(it)
                if it % 2 == 0:
                    stage_load(it // 2 + 1)  # prefetch next pair's inputs
                stage_a2(it)
            if it >= 4:
                stage_c(it - 4)
            if 3 <= it <= H + 2:
                stage_pv(it - 3)
            if 1 <= it <= H:
                stage_b1(it - 1)
    nc.compile()
    return nc


_NC_CACHE = None
_RUN_KW = {}
_LAST_RES = None


def _get_nc():
    global _NC_CACHE
    if _NC_CACHE is None:
        _NC_CACHE = build_nc()
    return _NC_CACHE


def prep_inputs(q, k, v1, v2, counter_attention_mask):
    """Host-side shard prep: transpose q/k, fold counter-mask into v."""
    import ml_dtypes

    bf = ml_dtypes.bfloat16
    q = np.asarray(q, dtype=np.float32)
    k = np.asarray(k, dtype=np.float32)
    v1 = np.asarray(v1, dtype=np.float32)
    v2 = np.asarray(v2, dtype=np.float32)
    cm = np.asarray(counter_attention_mask)
    notcm = (cm == 0).astype(np.float32)  # [B, S]

    maps = []
    for b in range(NCORES):
        ncm = notcm[b]                          # [S]
        v1m = v1[b] * ncm[:, None]              # [S, D]
        v2m = v2[b] * ncm[:, None]
        vx = np.zeros((S, H, HV), dtype=np.float32)
        vx[:, :, 0:DK] = v1m.reshape(S, H, DK)
        vx[:, :, DK : 2 * DK] = v2m.reshape(S, H, DK)
        vx[:, :, 2 * DK] = ncm[:, None]

        # corr[qb, h] = colsumAll(v) - colsum_{j<W}(vm); z col: S - countW
        ch1 = v1m.reshape(NQB, P, D).sum(axis=1)      # [8, D]
        ch2 = v2m.reshape(NQB, P, D).sum(axis=1)
        chn = ncm.reshape(NQB, P).sum(axis=1)         # [8]
        pre1 = np.cumsum(ch1, axis=0)                 # prefix sums per W
        pre2 = np.cumsum(ch2, axis=0)
        pren = np.cumsum(chn, axis=0)
        corr = np.zeros((NQB, H, HV), dtype=np.float32)
        corr[:, :, 0:DK] = (v1[b].sum(axis=0) - pre1).reshape(NQB, H, DK)
        corr[:, :, DK : 2 * DK] = (v2[b].sum(axis=0) - pre2).reshape(NQB, H, DK)
        corr[:, :, 2 * DK] = (S - pren)[:, None]

        maps.append({
            "qT": np.ascontiguousarray(q[b].astype(np.float16).T),
            "kT": np.ascontiguousarray(k[b].astype(np.float16).T),
            "vx": vx.reshape(S, H * HV).astype(bf),
            "corr": corr.reshape(1, NQB * H * HV).astype(np.float16),
        })
    return maps


def kernel(q, k, v1, v2, counter_attention_mask):
    global _LAST_RES
    from concourse.bass_utils import run_bass_kernel_spmd

    in_maps = prep_inputs(q, k, v1, v2, counter_attention_mask)
    nc = _get_nc()
    res = run_bass_kernel_spmd(nc, in_maps, list(range(NCORES)), **_RUN_KW)
    _LAST_RES = res
    out1 = np.stack([res.results[b]["out1"].astype(np.float32)
                     for b in range(NCORES)])
    out2 = np.stack([res.results[b]["out2"].astype(np.float32)
                     for b in range(NCORES)])
    return out1, out2
